# revision 70
# baseline (speedup 1.0000x reference)
"""Causal multi-head self-attention kernel for Trainium2 (Bass/Tile), 8 cores.

Problem: B=4, T=2048, D=1024, H=16 (DH=64), fp32, causal mask, no padding.

Sharding (8 cores): core c = 2*b + hg handles batch b = c//2 and head-group
hg = c%2 (8 of 16 heads). Each core computes its QKV projection slice, causal
attention for its heads, and a partial output projection over its 512
features. Host sums the two partial projections per batch.

v2 design (cost-model-driven; all inputs bf16-quantized on host):
  - x is fed pre-transposed from host (xT [D,T] bf16) so no PE transposes
    or staging copies are needed for the QKV projections; w_qkv/w_out are
    fed bf16 too (halves startup DMA).
  - q/k/v projections accumulate in PSUM over 8 K-chunks, outputs stored
    bf16: qkT pair tiles [128, T] feature-major, V token-major in vext
    [128 keys, 8*(64+1)] with a ones column per head (softmax denominator).
  - attention is a flat stream over (qb: 512-query block, h: single head):
    per key-block kb, S^T = K_h Q_h^T is ONE matmul into a single PSUM bank
    (the hardware allows only one accumulation group per 2KB bank — no
    sharing); P = exp(S^T/8) on ACT -> bf16; causal masking of diagonal
    blocks via gpsimd affine_select on P; P.V flipped token-major:
    out[128 queries, 65] with moving dim 65 (half the rows of the
    feature-major form), the four 128-query sub-blocks accumulating in four
    dedicated banks at 512-float offsets of one otx tile; normalization is
    a per-partition scalar multiply (DVE reciprocal [128,1] +
    tensor_scalar_mul), no partition broadcasts.
  - O (token-major bf16) is PE-transposed back to feature-major OTf for the
    output projection y = O @ W_out.
  - The exp chain makes attention ACT-bound; QKV-projection (in 256-token
    chunks with data-dependence deadlines) and output-projection matmuls
    are interleaved as "filler" PE work inside the attention kb loop
    (generator-based scheduler with deadline pacing + spread-to-end
    reservation) so the PE rarely idles behind ACT. P.V lags one kb behind
    S^T, and each group's normalize is deferred past the next group's first
    exp, to keep both engines streaming across group boundaries.
"""
import os
import numpy as np

B, T, D, H = 4, 2048, 1024, 16
DH = 64
HL = 8            # heads per core
FL = HL * DH      # 512 local features
NCORES = 8
DC = D // 128     # 8 contraction chunks
NTB = T // 512    # 4 big token blocks
NKB = T // 128    # 16 key blocks
NHQ = T // 256    # 8 query half-blocks
NG = T // 128     # 16 query 128-blocks
SCALE = 1.0 / 8.0  # 1/sqrt(DH)

_PROGRAM_CACHE = {}
LAST_RESULTS = None


def _build_program(is_causal: bool):
    import concourse.mybir as mybir
    import concourse.tile as tile
    from concourse import bacc

    F32 = mybir.dt.float32
    F32R = mybir.dt.float32r
    BF16 = mybir.dt.bfloat16
    AF = mybir.ActivationFunctionType
    ALU = mybir.AluOpType

    nc = bacc.Bacc("TRN2", target_bir_lowering=False, debug=False)
    xT = nc.dram_tensor("xT", [D, T], BF16, kind="ExternalInput").ap()
    w_qkv = nc.dram_tensor("w_qkv", [D, 3 * FL], BF16, kind="ExternalInput").ap()
    w_out = nc.dram_tensor("w_out", [FL, D], BF16, kind="ExternalInput").ap()
    y = nc.dram_tensor("y", [T, D], F32, kind="ExternalOutput").ap()

    from contextlib import ExitStack
    with tile.TileContext(nc) as tc:
        with ExitStack() as ctx:
            constp = ctx.enter_context(tc.tile_pool(name="const", bufs=1))
            wqp = ctx.enter_context(tc.tile_pool(name="wq", bufs=1))
            wop = ctx.enter_context(tc.tile_pool(name="wo", bufs=1))
            qkTp = ctx.enter_context(tc.tile_pool(name="qkTp", bufs=1))
            vextp = ctx.enter_context(tc.tile_pool(name="vextp", bufs=1))
            otfp = ctx.enter_context(tc.tile_pool(name="otfp", bufs=1))
            xtp = ctx.enter_context(tc.tile_pool(name="xtp", bufs=2))
            ptp = ctx.enter_context(tc.tile_pool(name="ptp", bufs=17))
            osp = ctx.enter_context(tc.tile_pool(name="osp", bufs=3))
            rcp = ctx.enter_context(tc.tile_pool(name="rcp", bufs=3))
            ysbp = ctx.enter_context(tc.tile_pool(name="ysbp", bufs=3))
            ps_st = ctx.enter_context(
                tc.tile_pool(name="ps_st", bufs=3, space="PSUM"))
            ps_ot = ctx.enter_context(
                tc.tile_pool(name="ps_ot", bufs=1, space="PSUM"))
            ps_qk = ctx.enter_context(
                tc.tile_pool(name="ps_qk", bufs=1, space="PSUM"))

            identity = constp.tile([128, 128], BF16)
            nc.gpsimd.memset(identity, 0.0)
            nc.gpsimd.affine_select(
                out=identity, in_=identity, compare_op=ALU.not_equal,
                fill=1.0, base=0, pattern=[[-1, 128]], channel_multiplier=1)

            # persistent tiles
            wqkv_r = [wqp.tile([128, 3 * FL], BF16, name=f"wqkvr{dc}")
                      for dc in range(DC)]
            wout_r = [wop.tile([128, D], BF16, name=f"woutr{fb}")
                      for fb in range(4)]
            # qkT[0..3]: q pairs, qkT[4..7]: k pairs; [2*64 dims, T] bf16
            qkT = [qkTp.tile([128, T], BF16, name=f"qkT{i}") for i in range(8)]
            # V_ext[kb]: [128 keys, 8 heads * (64 dims + ones col)] bf16
            vext = [vextp.tile([128, HL * 65], BF16, name=f"vext{i}")
                    for i in range(NKB)]
            # OTf: normalized attention output, feature-major [128 f, T] bf16
            OTf = [otfp.tile([128, T], BF16, name=f"OTf{i}") for i in range(4)]

            for kb in range(NKB):
                nc.gpsimd.memset(
                    vext[kb].rearrange("p (h c) -> p h c", h=HL)[:, :, 64:65],
                    1.0)

            # ---- prologue DMAs: per-dc interleave so ph2(0) can trickle ----
            xt_tiles = {}  # (dc, tb) -> tile

            def dma_xt_one(dc, tb):
                t = xtp.tile([128, 512], BF16, name=f"xt{dc}", tag=f"xt{dc}")
                nc.sync.dma_start(
                    t, xT[dc * 128:(dc + 1) * 128,
                          tb * 512:(tb + 1) * 512])
                xt_tiles[(dc, tb)] = t

            def dma_xt(tb):
                for dc in range(DC):
                    dma_xt_one(dc, tb)

            for dc in range(DC):
                nc.sync.dma_start(wqkv_r[dc][:, 0:2 * FL],
                                  w_qkv[dc * 128:(dc + 1) * 128, 0:2 * FL])
                dma_xt_one(dc, 0)
            for dc in range(DC):
                nc.sync.dma_start(
                    wqkv_r[dc][:, 2 * FL:3 * FL],
                    w_qkv[dc * 128:(dc + 1) * 128, 2 * FL:3 * FL])
            for fb in range(4):
                nc.sync.dma_start(wout_r[fb], w_out[fb * 128:(fb + 1) * 128, :])

            # ---- phase generators (filler quanta yield approx PE ns) ----
            def gen_ph2(c):
                """QKV projections for 256-token chunk c (tokens 256c..)."""
                tb, o = c // 2, (c % 2) * 256
                # q,k: feature-major [128 f, 256 t]
                for fb in range(8):
                    pqk = ps_qk.tile([128, 512], F32, name="pqk", tag="fil")
                    for dc in range(DC):
                        nc.tensor.matmul(
                            pqk[:, 0:256],
                            wqkv_r[dc][:, fb * 128:(fb + 1) * 128],
                            xt_tiles[(dc, tb)][:, o:o + 256],
                            start=(dc == 0), stop=(dc == DC - 1))
                        if dc == DC - 1:
                            nc.vector.tensor_copy(
                                qkT[fb][:, c * 256:(c + 1) * 256],
                                pqk[:, 0:256])
                        yield 107
                # v: token-major [128 t, 512 f] -> vext
                for ti in range(2):
                    ts_ = (c % 2) * 2 + ti
                    pv = ps_qk.tile([128, 512], F32, name="pv", tag="fil")
                    for dc in range(DC):
                        nc.tensor.matmul(
                            pv,
                            xt_tiles[(dc, tb)][:, ts_ * 128:(ts_ + 1) * 128],
                            wqkv_r[dc][:, 2 * FL:3 * FL],
                            start=(dc == 0), stop=(dc == DC - 1))
                        if dc == DC - 1:
                            kb = tb * 4 + ts_
                            nc.vector.tensor_copy(
                                vext[kb].rearrange("p (h c) -> p h c",
                                                   h=HL)[:, :, 0:64],
                                pv.rearrange("p (h c) -> p h c", h=HL))
                        yield 215
                if c % 2 == 1 and tb + 1 < NTB:
                    dma_xt(tb + 1)
                    yield 0

            def gen_tr(qb, fb, osg):
                """Transpose head-pair fb's slices of the four Ostage blocks
                of qb into OTf[fb]; available right after both heads of the
                pair finished their normalize."""
                for gl in range(4):
                    g = 4 * qb + gl
                    ptr = ps_qk.tile([128, 128], BF16, name="ptr",
                                     tag="fil")
                    nc.tensor.transpose(
                        ptr, osg[gl][:, fb * 128:(fb + 1) * 128], identity)
                    nc.vector.tensor_copy(
                        OTf[fb][:, g * 128:(g + 1) * 128], ptr)
                    yield 107

            def gen_ph4(qb):
                """Output projection + store for the four query blocks of
                qb (transposes for all head pairs already emitted)."""
                for gl in range(4):
                    g = 4 * qb + gl
                    ysb = ysbp.tile([128, D], F32, name="ysb", tag="ysb")
                    for nb in range(2):
                        py = ps_qk.tile([128, 512], F32, name="py", tag="fil")
                        for fb in range(4):
                            nc.tensor.matmul(
                                py, OTf[fb][:, g * 128:(g + 1) * 128],
                                wout_r[fb][:, nb * 512:(nb + 1) * 512],
                                start=(fb == 0), stop=(fb == 3))
                            if fb == 1:
                                yield 430
                        nc.vector.tensor_copy(
                            ysb[:, nb * 512:(nb + 1) * 512], py)
                        nc.sync.dma_start(
                            y[g * 128:(g + 1) * 128,
                              nb * 512:(nb + 1) * 512],
                            ysb[:, nb * 512:(nb + 1) * 512])
                        yield 430

            fillers = []       # [gen, ...] FIFO
            rem_ns = {}        # gen -> remaining PE ns (estimate)
            rem_tot = [0.0]    # total remaining filler PE ns
            deadline = {}      # gen -> iteration index by which it must be
                               # fully emitted (data-dependence deadline)
            hard_due = {}      # gen -> deadline (only gens attention reads)
            not_before = {}    # gen -> iteration before which it is held in
                               # reserve (to fill the late ACT-bound windows)
            cur_it_box = [0]

            def add_filler(g, est, dl=None, hard=False, nb=None):
                rem_ns[g] = est
                rem_tot[0] += est
                if dl is not None:
                    deadline[g] = dl
                    if hard:
                        hard_due[g] = dl
                if nb is not None:
                    not_before[g] = nb
                fillers.append(g)

            def pull_one():
                """Advance the head filler generator; return emitted PE ns."""
                g = fillers[0]
                try:
                    n = next(g)
                    rem_ns[g] = rem_ns.get(g, 0) - n
                    rem_tot[0] -= n
                    return n
                except StopIteration:
                    fillers.pop(0)
                    hard_due.pop(g, None)
                    return 0

            def emit_filler(ns):
                while ns > 0 and fillers:
                    if cur_it_box[0] < not_before.get(fillers[0], 0):
                        break
                    ns -= pull_one()
                return ns

            def drain_gen(g):
                for _ in g:
                    pass

            def ensure_due(cur_it):
                """Force-drain filler generators whose data is required by
                the current iteration, so no attention read of qkT/vext
                precedes its writer in program order. Deadline gens sit at
                the head of the FIFO queue."""
                while fillers and any(dl <= cur_it for dl in hard_due.values()):
                    pull_one()

            # ---- prologue compute: chunk 0 only (q,k for tokens 0..255 and
            # v for kb0,kb1) — exactly what hq0 needs. dc-paced in two passes
            # of 4 psum accumulators (tags fil, st, st, st) so the first
            # matmuls only wait their own dc chunk of the weight DMA.
            PH2_NS = 64 * 107 + 16 * 215
            # full ph2 of token block 0 (qb0 needs qkT cols 0:512 and
            # vext[0..3]); 4 accumulators borrowed from the otx banks so
            # each dc chunk of the weight DMA can be consumed as it arrives
            pacc = ps_ot.tile([128, 2048], F32, name="pacc", tag="otx")
            for half in range(2):
                for dc in range(DC):
                    for fi in range(4):
                        fb = half * 4 + fi
                        nc.tensor.matmul(
                            pacc[:, fi * 512:(fi + 1) * 512],
                            wqkv_r[dc][:, fb * 128:(fb + 1) * 128],
                            xt_tiles[(dc, 0)],
                            start=(dc == 0), stop=(dc == DC - 1))
                for fi in range(4):
                    fb = half * 4 + fi
                    nc.vector.tensor_copy(
                        qkT[fb][:, 0:512], pacc[:, fi * 512:(fi + 1) * 512])
            for ts_ in range(4):
                pv = ps_qk.tile([128, 512], F32, name="pv", tag="fil")
                for dc in range(DC):
                    nc.tensor.matmul(
                        pv, xt_tiles[(dc, 0)][:, ts_ * 128:(ts_ + 1) * 128],
                        wqkv_r[dc][:, 2 * FL:3 * FL],
                        start=(dc == 0), stop=(dc == DC - 1))
                nc.vector.tensor_copy(
                    vext[ts_].rearrange("p (h c) -> p h c", h=HL)[:, :, 0:64],
                    pv.rearrange("p (h c) -> p h c", h=HL))
            dma_xt(1)

            # iteration bookkeeping for deadline pacing: group (qb, h)
            # starts at iteration gstart[(qb, h)]
            gstart = {}
            it = 0
            for qb_ in range(NQB):
                for h_ in range(HL):
                    gstart[(qb_, h_)] = it
                    it += (4 * qb_ + 4) if is_causal else NKB
            IT_TOTAL = it

            for c in range(2, 2 * NTB):
                # chunk c must be emitted before the first group of its qb
                add_filler(gen_ph2(c), PH2_NS, gstart[(c // 2, 0)],
                           hard=True)

            # ---- attention: flat stream over (qb: 512-query block, h:
            # single head) with cross-group PV/normalize lag and
            # deadline-paced filler. One PSUM accumulation group per bank
            # throughout: S^T is one matmul per kb into a 1-bank st tile;
            # P.V accumulates the 4 query sub-blocks in 4 dedicated banks
            # (512-float offsets of the otx tile). ----
            pend = None          # (kb, pt, otx, qb, h)
            pending_norm = None  # closure finishing previous group

            def make_norm(otx, osg, h):
                def norm():
                    rc = rcp.tile([128, 4], F32, name="rc", tag="rc")
                    for gl in range(4):
                        o0 = gl * 512
                        nc.vector.reciprocal(
                            rc[:, gl:gl + 1], otx[:, o0 + 64:o0 + 65])
                        nc.vector.tensor_scalar_mul(
                            osg[gl][:, h * 64:(h + 1) * 64],
                            otx[:, o0:o0 + 64],
                            rc[:, gl:gl + 1])
                return norm

            cur_it = 0
            for qb in range(NQB):
                for h in range(HL):
                    hp = h // 2
                    par = h % 2
                    nkb = 4 * qb + 4 if is_causal else NKB
                    ensure_due(cur_it)
                    otx = ps_ot.tile([128, 2048], F32, name="otx", tag="otx")
                    if h == 0:
                        osg = [osp.tile([128, 512], BF16, name=f"os{gl}",
                                        tag=f"os{gl}") for gl in range(4)]
                    for kb in range(nkb):
                        diag = is_causal and kb >= 4 * qb
                        j = kb - 4 * qb if diag else 0
                        c0 = 128 * j
                        w = 512 - c0
                        st = ps_st.tile([128, 512], F32, name="st", tag="st")
                        nc.tensor.matmul(
                            st[:, c0:512],
                            qkT[4 + hp][par * 64:(par + 1) * 64,
                                        kb * 128:(kb + 1) * 128],
                            qkT[hp][par * 64:(par + 1) * 64,
                                    qb * 512 + c0:(qb + 1) * 512],
                            start=True, stop=True)
                        pt = ptp.tile([128, 512], BF16, name="pt", tag="pt")
                        nc.scalar.activation(
                            pt[:, c0:512], st[:, c0:512], AF.Exp, scale=SCALE)
                        if diag:
                            # zero where key p > query col (gpsimd)
                            nc.gpsimd.affine_select(
                                out=pt[:, c0:512], in_=pt[:, c0:512],
                                compare_op=ALU.is_ge, fill=0.0,
                                base=0,
                                pattern=[[1, w]],
                                channel_multiplier=-1)
                        # P.V for previous kb (its exp is finished); may
                        # belong to the previous group
                        if pend is not None:
                            _emit_pv(nc, pend[2], pend[0], pend[1],
                                     pend[3], pend[4], vext)
                            if pend[3] != qb or pend[4] != h:
                                pending_norm()
                                pending_norm = None
                        pend = (kb, pt, otx, qb, h)
                        # filler while ACT computes exp(kb). Policy: at least
                        # the head gen's deadline rate; otherwise the ACT-PE
                        # gap, but no more than the spread-to-end rate (so
                        # work remains for the final ACT-bound windows).
                        quota_head = 0.0
                        if fillers:
                            gh = fillers[0]
                            dl = deadline.get(gh)
                            if dl is not None:
                                slack = dl - cur_it
                                r = float(rem_ns.get(gh, 0))
                                quota_head = r / slack if slack > 1 else r
                        left = max(1, IT_TOTAL - cur_it)
                        quota_all = rem_tot[0] / left
                        cur_it_box[0] = cur_it
                        gap = 0.85 * (w * 1.01 + 120 - (w + 170) * 0.42)
                        emit_filler(int(max(min(quota_head, 2500),
                                            min(gap, quota_all))))
                        cur_it += 1
                    pending_norm = make_norm(otx, osg, h)
                    if h % 2 == 1:
                        add_filler(gen_tr(qb, h // 2, osg), 4 * 107)
                add_filler(gen_ph4(qb), 4 * 4 * 430)

            # ---- epilogue ----
            if pend is not None:
                _emit_pv(nc, pend[2], pend[0], pend[1], pend[3], pend[4],
                         vext)
                pending_norm()
            while fillers:
                drain_gen(fillers.pop(0))

    nc.compile()
    return nc


def _emit_pv(nc, otx, kb, pt, qb, h, vext):
    """P.V flipped token-major: out [128 queries, 65] per 128-query
    sub-block gl. The four accumulators sit at 512-float offsets of the otx
    tile so each accumulation group owns a full PSUM bank."""
    for gl in range(4):
        g = 4 * qb + gl
        if kb > g:
            continue
        o0 = gl * 512
        nc.tensor.matmul(
            otx[:, o0:o0 + 65],
            pt[:, gl * 128:(gl + 1) * 128],
            vext[kb][:, h * 65:(h + 1) * 65],
            start=(kb == 0),
            stop=(kb == g))


def _get_program(is_causal: bool):
    key = ("causal" if is_causal else "full")
    if key not in _PROGRAM_CACHE:
        _PROGRAM_CACHE[key] = _build_program(is_causal)
    return _PROGRAM_CACHE[key]


def _numpy_fallback(x, W_qkv, W_out, attn_mask, key_padding_mask):
    import math
    qkv = x @ W_qkv
    q, k, v = np.split(qkv, 3, axis=-1)
    q = q.reshape(B, T, H, DH).transpose(0, 2, 1, 3)
    k = k.reshape(B, T, H, DH).transpose(0, 2, 1, 3)
    v = v.reshape(B, T, H, DH).transpose(0, 2, 1, 3)
    scores = np.einsum('bhqd,bhkd->bhqk', q, k) / math.sqrt(DH)
    scores = np.where(attn_mask[None, None, :, :], -np.inf, scores)
    scores = np.where(key_padding_mask[:, None, None, :], -np.inf, scores)
    scores = scores - scores.max(axis=-1, keepdims=True)
    attn = np.exp(scores)
    attn = attn / attn.sum(axis=-1, keepdims=True)
    out = np.einsum('bhqk,bhkd->bhqd', attn, v)
    out = out.transpose(0, 2, 1, 3).reshape(B, T, D)
    return (out @ W_out).astype(np.float32)


def build_in_maps(inputs):
    import ml_dtypes
    BF = ml_dtypes.bfloat16
    x = np.asarray(inputs["x"], dtype=np.float32)
    W_qkv = np.asarray(inputs["W_qkv"], dtype=np.float32)
    W_out = np.asarray(inputs["W_out"], dtype=np.float32)
    in_maps = []
    xt_cache = {}
    for c in range(NCORES):
        b, hg = c // 2, c % 2
        cols = slice(hg * FL, (hg + 1) * FL)
        w_qkv_local = np.ascontiguousarray(np.concatenate(
            [W_qkv[:, D * i:D * (i + 1)][:, cols] for i in range(3)],
            axis=1).astype(BF))
        w_out_local = np.ascontiguousarray(W_out[cols, :].astype(BF))
        if b not in xt_cache:
            xt_cache[b] = np.ascontiguousarray(x[b].T.astype(BF))
        in_maps.append({"xT": xt_cache[b], "w_qkv": w_qkv_local,
                        "w_out": w_out_local})
    return in_maps


def kernel(x, W_qkv, W_out, attn_mask, key_padding_mask):
    global LAST_RESULTS
    x = np.ascontiguousarray(np.asarray(x, dtype=np.float32))
    W_qkv = np.ascontiguousarray(np.asarray(W_qkv, dtype=np.float32))
    W_out = np.ascontiguousarray(np.asarray(W_out, dtype=np.float32))
    attn_mask = np.asarray(attn_mask).astype(bool)
    if attn_mask.ndim > 2:  # tolerate leading singleton dims
        attn_mask = attn_mask.reshape(attn_mask.shape[-2], attn_mask.shape[-1])
    key_padding_mask = np.asarray(key_padding_mask).astype(bool)
    if key_padding_mask.ndim > 2:
        key_padding_mask = key_padding_mask.reshape(
            key_padding_mask.shape[-2], key_padding_mask.shape[-1])

    causal = np.array_equal(
        attn_mask, np.triu(np.ones((T, T), dtype=bool), k=1))
    if key_padding_mask.any() or not causal:
        return _numpy_fallback(x, W_qkv, W_out, attn_mask, key_padding_mask)

    os.environ["BASS_NEVER_TRACE"] = "1"  # axon NTFF hook unavailable here
    from concourse.bass_utils import run_bass_kernel_spmd

    nc = _get_program(causal)
    in_maps = build_in_maps(
        {"x": x, "W_qkv": W_qkv, "W_out": W_out})

    res = run_bass_kernel_spmd(nc, in_maps, core_ids=list(range(NCORES)))
    LAST_RESULTS = res
    out = np.zeros((B, T, D), dtype=np.float32)
    for c in range(NCORES):
        out[c // 2] += res.results[c]["y"]
    return out


# revision 73
# speedup vs baseline: 1.1605x; 1.1605x over previous
"""Causal multi-head self-attention kernel for Trainium2 (Bass/Tile), 8 cores.

Problem: B=4, T=2048, D=1024, H=16 (DH=64), fp32, causal mask, no padding.

Sharding (8 cores): core c = 2*b + hg handles batch b = c//2 and head-group
hg = c%2 (8 of 16 heads). Each core computes its QKV projection slice, causal
attention for its heads, and a partial output projection over its 512
features. Host sums the two partial projections per batch.

v2 design (cost-model-driven; all inputs bf16-quantized on host):
  - x is fed pre-transposed from host (xT [D,T] bf16) so no PE transposes
    or staging copies are needed for the QKV projections; w_qkv/w_out are
    fed bf16 too (halves startup DMA).
  - q/k/v projections accumulate in PSUM over 8 K-chunks, outputs stored
    bf16: qkT pair tiles [128, T] feature-major, V token-major in vext
    [128 keys, 8*(64+1)] with a ones column per head (softmax denominator).
  - attention is a flat stream over (qb: 512-query block, h: single head):
    per key-block kb, S^T = K_h Q_h^T is ONE matmul into a single PSUM bank
    (the hardware allows only one accumulation group per 2KB bank — no
    sharing); P = exp(S^T/8) on ACT -> bf16; causal masking of diagonal
    blocks via gpsimd affine_select on P; P.V flipped token-major:
    out[128 queries, 65] with moving dim 65 (half the rows of the
    feature-major form), the four 128-query sub-blocks accumulating in four
    dedicated banks at 512-float offsets of one otx tile; normalization is
    a per-partition scalar multiply (DVE reciprocal [128,1] +
    tensor_scalar_mul), no partition broadcasts.
  - O (token-major bf16) is PE-transposed back to feature-major OTf for the
    output projection y = O @ W_out.
  - The exp chain makes attention ACT-bound; QKV-projection (in 256-token
    chunks with data-dependence deadlines) and output-projection matmuls
    are interleaved as "filler" PE work inside the attention kb loop
    (generator-based scheduler with deadline pacing + spread-to-end
    reservation) so the PE rarely idles behind ACT. P.V lags one kb behind
    S^T, and each group's normalize is deferred past the next group's first
    exp, to keep both engines streaming across group boundaries.
"""
import os
import numpy as np

B, T, D, H = 4, 2048, 1024, 16
DH = 64
HL = 8            # heads per core
FL = HL * DH      # 512 local features
NCORES = 8
DC = D // 128     # 8 contraction chunks
NTB = T // 512    # 4 big token blocks
NKB = T // 128    # 16 key blocks
NHQ = T // 256    # 8 query half-blocks
NG = T // 128     # 16 query 128-blocks
SCALE = 1.0 / 8.0  # 1/sqrt(DH)

_PROGRAM_CACHE = {}
LAST_RESULTS = None


def _build_program(is_causal: bool):
    import concourse.mybir as mybir
    import concourse.tile as tile
    from concourse import bacc

    F32 = mybir.dt.float32
    F32R = mybir.dt.float32r
    BF16 = mybir.dt.bfloat16
    AF = mybir.ActivationFunctionType
    ALU = mybir.AluOpType

    nc = bacc.Bacc("TRN2", target_bir_lowering=False, debug=False)
    xT = nc.dram_tensor("xT", [D, T], BF16, kind="ExternalInput").ap()
    w_qkv = nc.dram_tensor("w_qkv", [D, 3 * FL], BF16, kind="ExternalInput").ap()
    w_out = nc.dram_tensor("w_out", [FL, D], BF16, kind="ExternalInput").ap()
    y = nc.dram_tensor("y", [T, D], F32, kind="ExternalOutput").ap()

    from contextlib import ExitStack
    with tile.TileContext(nc) as tc:
        with ExitStack() as ctx:
            constp = ctx.enter_context(tc.tile_pool(name="const", bufs=1))
            wqp = ctx.enter_context(tc.tile_pool(name="wq", bufs=1))
            wop = ctx.enter_context(tc.tile_pool(name="wo", bufs=1))
            qkTp = ctx.enter_context(tc.tile_pool(name="qkTp", bufs=1))
            vextp = ctx.enter_context(tc.tile_pool(name="vextp", bufs=1))
            otfp = ctx.enter_context(tc.tile_pool(name="otfp", bufs=1))
            xtp = ctx.enter_context(tc.tile_pool(name="xtp", bufs=2))
            ptp = ctx.enter_context(tc.tile_pool(name="ptp", bufs=17))
            osp = ctx.enter_context(tc.tile_pool(name="osp", bufs=3))
            rcp = ctx.enter_context(tc.tile_pool(name="rcp", bufs=3))
            ysbp = ctx.enter_context(tc.tile_pool(name="ysbp", bufs=3))
            ps_st = ctx.enter_context(
                tc.tile_pool(name="ps_st", bufs=3, space="PSUM"))
            ps_ot = ctx.enter_context(
                tc.tile_pool(name="ps_ot", bufs=1, space="PSUM"))
            ps_qk = ctx.enter_context(
                tc.tile_pool(name="ps_qk", bufs=1, space="PSUM"))

            identity = constp.tile([128, 128], BF16)
            nc.gpsimd.memset(identity, 0.0)
            nc.gpsimd.affine_select(
                out=identity, in_=identity, compare_op=ALU.not_equal,
                fill=1.0, base=0, pattern=[[-1, 128]], channel_multiplier=1)

            # persistent tiles
            wqkv_r = [wqp.tile([128, 3 * FL], BF16, name=f"wqkvr{dc}")
                      for dc in range(DC)]
            wout_r = [wop.tile([128, D], BF16, name=f"woutr{fb}")
                      for fb in range(4)]
            # qkT[0..3]: q pairs, qkT[4..7]: k pairs; [2*64 dims, T] bf16
            qkT = [qkTp.tile([128, T], BF16, name=f"qkT{i}") for i in range(8)]
            # V_ext[kb]: [128 keys, 8 heads * (64 dims + ones col)] bf16
            vext = [vextp.tile([128, HL * 65], BF16, name=f"vext{i}")
                    for i in range(NKB)]
            # OTf: normalized attention output, feature-major [128 f, T] bf16
            OTf = [otfp.tile([128, T], BF16, name=f"OTf{i}") for i in range(4)]

            for kb in range(NKB):
                nc.gpsimd.memset(
                    vext[kb].rearrange("p (h c) -> p h c", h=HL)[:, :, 64:65],
                    1.0)

            # ---- prologue DMAs: per-dc interleave so ph2(0) can trickle ----
            xt_tiles = {}  # (dc, tb) -> tile

            def dma_xt_one(dc, tb):
                t = xtp.tile([128, 512], BF16, name=f"xt{dc}", tag=f"xt{dc}")
                nc.sync.dma_start(
                    t, xT[dc * 128:(dc + 1) * 128,
                          tb * 512:(tb + 1) * 512])
                xt_tiles[(dc, tb)] = t

            def dma_xt(tb):
                for dc in range(DC):
                    dma_xt_one(dc, tb)

            for dc in range(DC):
                nc.sync.dma_start(wqkv_r[dc][:, 0:2 * FL],
                                  w_qkv[dc * 128:(dc + 1) * 128, 0:2 * FL])
                dma_xt_one(dc, 0)
            for dc in range(DC):
                nc.sync.dma_start(
                    wqkv_r[dc][:, 2 * FL:3 * FL],
                    w_qkv[dc * 128:(dc + 1) * 128, 2 * FL:3 * FL])
            for fb in range(4):
                nc.sync.dma_start(wout_r[fb], w_out[fb * 128:(fb + 1) * 128, :])

            # ---- phase generators (filler quanta yield approx PE ns) ----
            def gen_ph2(c):
                """QKV projections for 256-token chunk c (tokens 256c..)."""
                tb, o = c // 2, (c % 2) * 256
                # q,k: feature-major [128 f, 256 t]
                for fb in range(8):
                    pqk = ps_qk.tile([128, 512], F32, name="pqk", tag="fil")
                    for dc in range(DC):
                        nc.tensor.matmul(
                            pqk[:, 0:256],
                            wqkv_r[dc][:, fb * 128:(fb + 1) * 128],
                            xt_tiles[(dc, tb)][:, o:o + 256],
                            start=(dc == 0), stop=(dc == DC - 1))
                        if dc == DC - 1:
                            nc.vector.tensor_copy(
                                qkT[fb][:, c * 256:(c + 1) * 256],
                                pqk[:, 0:256])
                        yield 107
                # v: token-major [128 t, 512 f] -> vext
                for ti in range(2):
                    ts_ = (c % 2) * 2 + ti
                    pv = ps_qk.tile([128, 512], F32, name="pv", tag="fil")
                    for dc in range(DC):
                        nc.tensor.matmul(
                            pv,
                            xt_tiles[(dc, tb)][:, ts_ * 128:(ts_ + 1) * 128],
                            wqkv_r[dc][:, 2 * FL:3 * FL],
                            start=(dc == 0), stop=(dc == DC - 1))
                        if dc == DC - 1:
                            kb = tb * 4 + ts_
                            nc.vector.tensor_copy(
                                vext[kb].rearrange("p (h c) -> p h c",
                                                   h=HL)[:, :, 0:64],
                                pv.rearrange("p (h c) -> p h c", h=HL))
                        yield 215
                if c % 2 == 1 and tb + 1 < NTB:
                    dma_xt(tb + 1)
                    yield 0

            def gen_tr(qb, fb, osg):
                """Transpose head-pair fb's slices of the four Ostage blocks
                of qb into OTf[fb]; available right after both heads of the
                pair finished their normalize."""
                for gl in range(4):
                    g = 4 * qb + gl
                    ptr = ps_qk.tile([128, 128], BF16, name="ptr",
                                     tag="fil")
                    nc.tensor.transpose(
                        ptr, osg[gl][:, fb * 128:(fb + 1) * 128], identity)
                    nc.vector.tensor_copy(
                        OTf[fb][:, g * 128:(g + 1) * 128], ptr)
                    yield 107

            def gen_ph4(qb):
                """Output projection + store for the four query blocks of
                qb (transposes for all head pairs already emitted)."""
                for gl in range(4):
                    g = 4 * qb + gl
                    ysb = ysbp.tile([128, D], F32, name="ysb", tag="ysb")
                    for nb in range(2):
                        py = ps_qk.tile([128, 512], F32, name="py", tag="fil")
                        for fb in range(4):
                            nc.tensor.matmul(
                                py, OTf[fb][:, g * 128:(g + 1) * 128],
                                wout_r[fb][:, nb * 512:(nb + 1) * 512],
                                start=(fb == 0), stop=(fb == 3))
                            if fb == 1:
                                yield 430
                        nc.vector.tensor_copy(
                            ysb[:, nb * 512:(nb + 1) * 512], py)
                        nc.sync.dma_start(
                            y[g * 128:(g + 1) * 128,
                              nb * 512:(nb + 1) * 512],
                            ysb[:, nb * 512:(nb + 1) * 512])
                        yield 430

            fillers = []       # [gen, ...] FIFO
            rem_ns = {}        # gen -> remaining PE ns (estimate)
            rem_tot = [0.0]    # total remaining filler PE ns
            deadline = {}      # gen -> iteration index by which it must be
                               # fully emitted (data-dependence deadline)
            hard_due = {}      # gen -> deadline (only gens attention reads)
            not_before = {}    # gen -> iteration before which it is held in
                               # reserve (to fill the late ACT-bound windows)
            cur_it_box = [0]

            def add_filler(g, est, dl=None, hard=False, nb=None):
                rem_ns[g] = est
                rem_tot[0] += est
                if dl is not None:
                    deadline[g] = dl
                    if hard:
                        hard_due[g] = dl
                if nb is not None:
                    not_before[g] = nb
                fillers.append(g)

            def pull_one():
                """Advance the head filler generator; return emitted PE ns."""
                g = fillers[0]
                try:
                    n = next(g)
                    rem_ns[g] = rem_ns.get(g, 0) - n
                    rem_tot[0] -= n
                    return n
                except StopIteration:
                    fillers.pop(0)
                    hard_due.pop(g, None)
                    return 0

            def emit_filler(ns):
                while ns > 0 and fillers:
                    if cur_it_box[0] < not_before.get(fillers[0], 0):
                        break
                    ns -= pull_one()
                return ns

            def drain_gen(g):
                for _ in g:
                    pass

            def ensure_due(cur_it):
                """Force-drain filler generators whose data is required by
                the current iteration, so no attention read of qkT/vext
                precedes its writer in program order. Deadline gens sit at
                the head of the FIFO queue."""
                while fillers and any(dl <= cur_it for dl in hard_due.values()):
                    pull_one()

            # ---- prologue compute: chunk 0 only (q,k for tokens 0..255 and
            # v for kb0,kb1) — exactly what hq0 needs. dc-paced in two passes
            # of 4 psum accumulators (tags fil, st, st, st) so the first
            # matmuls only wait their own dc chunk of the weight DMA.
            PH2_NS = 64 * 107 + 16 * 215
            # full ph2 of token block 0 (qb0 needs qkT cols 0:512 and
            # vext[0..3]); 4 accumulators borrowed from the otx banks so
            # each dc chunk of the weight DMA can be consumed as it arrives
            pacc = ps_ot.tile([128, 2048], F32, name="pacc", tag="otx")
            for half in range(2):
                for dc in range(DC):
                    for fi in range(4):
                        fb = half * 4 + fi
                        nc.tensor.matmul(
                            pacc[:, fi * 512:(fi + 1) * 512],
                            wqkv_r[dc][:, fb * 128:(fb + 1) * 128],
                            xt_tiles[(dc, 0)],
                            start=(dc == 0), stop=(dc == DC - 1))
                for fi in range(4):
                    fb = half * 4 + fi
                    nc.vector.tensor_copy(
                        qkT[fb][:, 0:512], pacc[:, fi * 512:(fi + 1) * 512])
            for ts_ in range(4):
                pv = ps_qk.tile([128, 512], F32, name="pv", tag="fil")
                for dc in range(DC):
                    nc.tensor.matmul(
                        pv, xt_tiles[(dc, 0)][:, ts_ * 128:(ts_ + 1) * 128],
                        wqkv_r[dc][:, 2 * FL:3 * FL],
                        start=(dc == 0), stop=(dc == DC - 1))
                nc.vector.tensor_copy(
                    vext[ts_].rearrange("p (h c) -> p h c", h=HL)[:, :, 0:64],
                    pv.rearrange("p (h c) -> p h c", h=HL))
            dma_xt(1)

            # iteration bookkeeping for deadline pacing: group (qb, h)
            # starts at iteration gstart[(qb, h)]
            gstart = {}
            it = 0
            for qb_ in range(NQB):
                for h_ in range(HL):
                    gstart[(qb_, h_)] = it
                    it += (4 * qb_ + 4) if is_causal else NKB
            IT_TOTAL = it

            for c in range(2, 2 * NTB):
                # chunk c must be emitted before the first group of its qb
                add_filler(gen_ph2(c), PH2_NS, gstart[(c // 2, 0)],
                           hard=True)

            # ---- attention: flat stream over (qb: 512-query block, h:
            # single head) with cross-group PV/normalize lag and
            # deadline-paced filler. One PSUM accumulation group per bank
            # throughout: S^T is one matmul per kb into a 1-bank st tile;
            # P.V accumulates the 4 query sub-blocks in 4 dedicated banks
            # (512-float offsets of the otx tile). ----
            pend = None          # (kb, pt, otx, qb, h)
            pending_norm = None  # closure finishing previous group

            def make_norm(otx, osg, h):
                def norm():
                    rc = rcp.tile([128, 4], F32, name="rc", tag="rc")
                    for gl in range(4):
                        o0 = gl * 512
                        nc.vector.reciprocal(
                            rc[:, gl:gl + 1], otx[:, o0 + 64:o0 + 65])
                        nc.vector.tensor_scalar_mul(
                            osg[gl][:, h * 64:(h + 1) * 64],
                            otx[:, o0:o0 + 64],
                            rc[:, gl:gl + 1])
                return norm

            cur_it = 0
            for qb in range(NQB):
                for h in range(HL):
                    hp = h // 2
                    par = h % 2
                    nkb = 4 * qb + 4 if is_causal else NKB
                    ensure_due(cur_it)
                    otx = ps_ot.tile([128, 2048], F32, name="otx", tag="otx")
                    if h == 0:
                        osg = [osp.tile([128, 512], BF16, name=f"os{gl}",
                                        tag=f"os{gl}") for gl in range(4)]
                    for kb in range(nkb):
                        diag = is_causal and kb >= 4 * qb
                        j = kb - 4 * qb if diag else 0
                        c0 = 128 * j
                        w = 512 - c0
                        st = ps_st.tile([128, 512], F32, name="st", tag="st")
                        nc.tensor.matmul(
                            st[:, c0:512],
                            qkT[4 + hp][par * 64:(par + 1) * 64,
                                        kb * 128:(kb + 1) * 128],
                            qkT[hp][par * 64:(par + 1) * 64,
                                    qb * 512 + c0:(qb + 1) * 512],
                            start=True, stop=True)
                        pt = ptp.tile([128, 512], BF16, name="pt", tag="pt")
                        nc.scalar.activation(
                            pt[:, c0:512], st[:, c0:512], AF.Exp, scale=SCALE)
                        if diag:
                            # zero where key p > query col (gpsimd)
                            nc.gpsimd.affine_select(
                                out=pt[:, c0:512], in_=pt[:, c0:512],
                                compare_op=ALU.is_ge, fill=0.0,
                                base=0,
                                pattern=[[1, w]],
                                channel_multiplier=-1)
                        # P.V for previous kb (its exp is finished); may
                        # belong to the previous group
                        if pend is not None:
                            _emit_pv(nc, pend[2], pend[0], pend[1],
                                     pend[3], pend[4], vext)
                            if pend[3] != qb or pend[4] != h:
                                pending_norm()
                                pending_norm = None
                        pend = (kb, pt, otx, qb, h)
                        # filler while ACT computes exp(kb). Policy: at least
                        # the head gen's deadline rate; otherwise the ACT-PE
                        # gap, but no more than the spread-to-end rate (so
                        # work remains for the final ACT-bound windows).
                        # EDF-style required rate over the queued
                        # deadline gens (cumulative demand / slack)
                        quota_head = 0.0
                        _cum = 0.0
                        for gh in fillers[:6]:
                            dl = deadline.get(gh)
                            _cum += float(rem_ns.get(gh, 0))
                            if dl is None:
                                break
                            slack = dl - cur_it
                            quota_head = max(
                                quota_head,
                                _cum / slack if slack > 1 else _cum)
                        left = max(1, IT_TOTAL - cur_it)
                        quota_all = rem_tot[0] / left
                        cur_it_box[0] = cur_it
                        gap = 0.85 * (w * 1.01 + 120 - (w + 170) * 0.42)
                        emit_filler(int(max(min(quota_head, 2500),
                                            min(gap, quota_all))))
                        cur_it += 1
                    pending_norm = make_norm(otx, osg, h)
                    if h % 2 == 1:
                        add_filler(gen_tr(qb, h // 2, osg), 4 * 107)
                add_filler(gen_ph4(qb), 4 * 4 * 430)

            # ---- epilogue ----
            if pend is not None:
                _emit_pv(nc, pend[2], pend[0], pend[1], pend[3], pend[4],
                         vext)
                pending_norm()
            while fillers:
                drain_gen(fillers.pop(0))

    nc.compile()
    return nc


def _emit_pv(nc, otx, kb, pt, qb, h, vext):
    """P.V flipped token-major: out [128 queries, 65] per 128-query
    sub-block gl. The four accumulators sit at 512-float offsets of the otx
    tile so each accumulation group owns a full PSUM bank."""
    for gl in range(4):
        g = 4 * qb + gl
        if kb > g:
            continue
        o0 = gl * 512
        nc.tensor.matmul(
            otx[:, o0:o0 + 65],
            pt[:, gl * 128:(gl + 1) * 128],
            vext[kb][:, h * 65:(h + 1) * 65],
            start=(kb == 0),
            stop=(kb == g))


def _get_program(is_causal: bool):
    key = ("causal" if is_causal else "full")
    if key not in _PROGRAM_CACHE:
        _PROGRAM_CACHE[key] = _build_program(is_causal)
    return _PROGRAM_CACHE[key]


def _numpy_fallback(x, W_qkv, W_out, attn_mask, key_padding_mask):
    import math
    qkv = x @ W_qkv
    q, k, v = np.split(qkv, 3, axis=-1)
    q = q.reshape(B, T, H, DH).transpose(0, 2, 1, 3)
    k = k.reshape(B, T, H, DH).transpose(0, 2, 1, 3)
    v = v.reshape(B, T, H, DH).transpose(0, 2, 1, 3)
    scores = np.einsum('bhqd,bhkd->bhqk', q, k) / math.sqrt(DH)
    scores = np.where(attn_mask[None, None, :, :], -np.inf, scores)
    scores = np.where(key_padding_mask[:, None, None, :], -np.inf, scores)
    scores = scores - scores.max(axis=-1, keepdims=True)
    attn = np.exp(scores)
    attn = attn / attn.sum(axis=-1, keepdims=True)
    out = np.einsum('bhqk,bhkd->bhqd', attn, v)
    out = out.transpose(0, 2, 1, 3).reshape(B, T, D)
    return (out @ W_out).astype(np.float32)


def build_in_maps(inputs):
    import ml_dtypes
    BF = ml_dtypes.bfloat16
    x = np.asarray(inputs["x"], dtype=np.float32)
    W_qkv = np.asarray(inputs["W_qkv"], dtype=np.float32)
    W_out = np.asarray(inputs["W_out"], dtype=np.float32)
    in_maps = []
    xt_cache = {}
    for c in range(NCORES):
        b, hg = c // 2, c % 2
        cols = slice(hg * FL, (hg + 1) * FL)
        w_qkv_local = np.ascontiguousarray(np.concatenate(
            [W_qkv[:, D * i:D * (i + 1)][:, cols] for i in range(3)],
            axis=1).astype(BF))
        w_out_local = np.ascontiguousarray(W_out[cols, :].astype(BF))
        if b not in xt_cache:
            xt_cache[b] = np.ascontiguousarray(x[b].T.astype(BF))
        in_maps.append({"xT": xt_cache[b], "w_qkv": w_qkv_local,
                        "w_out": w_out_local})
    return in_maps


def kernel(x, W_qkv, W_out, attn_mask, key_padding_mask):
    global LAST_RESULTS
    x = np.ascontiguousarray(np.asarray(x, dtype=np.float32))
    W_qkv = np.ascontiguousarray(np.asarray(W_qkv, dtype=np.float32))
    W_out = np.ascontiguousarray(np.asarray(W_out, dtype=np.float32))
    attn_mask = np.asarray(attn_mask).astype(bool)
    if attn_mask.ndim > 2:  # tolerate leading singleton dims
        attn_mask = attn_mask.reshape(attn_mask.shape[-2], attn_mask.shape[-1])
    key_padding_mask = np.asarray(key_padding_mask).astype(bool)
    if key_padding_mask.ndim > 2:
        key_padding_mask = key_padding_mask.reshape(
            key_padding_mask.shape[-2], key_padding_mask.shape[-1])

    causal = np.array_equal(
        attn_mask, np.triu(np.ones((T, T), dtype=bool), k=1))
    if key_padding_mask.any() or not causal:
        return _numpy_fallback(x, W_qkv, W_out, attn_mask, key_padding_mask)

    os.environ["BASS_NEVER_TRACE"] = "1"  # axon NTFF hook unavailable here
    from concourse.bass_utils import run_bass_kernel_spmd

    nc = _get_program(causal)
    in_maps = build_in_maps(
        {"x": x, "W_qkv": W_qkv, "W_out": W_out})

    res = run_bass_kernel_spmd(nc, in_maps, core_ids=list(range(NCORES)))
    LAST_RESULTS = res
    out = np.zeros((B, T, D), dtype=np.float32)
    for c in range(NCORES):
        out[c // 2] += res.results[c]["y"]
    return out
